# revision 27
# baseline (speedup 1.0000x reference)
"""GAT encoder (3 GAT layers: 256-hid 4-head concat + mu/logvar 128) on 8 trn2 cores.

Strategy (dst-range node sharding, per sharding_hint):
 - Host sorts edges by dst, buckets per core (2500 dst nodes each), pads each
   128-node block to TPB tiles of 128 edges.
 - Tile plan is JIT-specialized to the actual graph (per-block tile count =
   max over cores of ceil(edges/128)).
 - Phase A (per core, redundant): xp = x @ W1 (bf16 matmul, batched bf16-only
   loads) for all nodes; ss1 logits reduced on-chip from the f32 PSUM
   (mult by replicated att row + grouped tensor_reduce); rows -> DRAM XPT
   (bf16 payload + f32 logit bytes). sd1 for own nodes via f32 matmul ->
   EXTD1 (separate from EXTD2 so L1 gathers never alias finalize writes).
 - L1 edge phase: dma_gather xp rows by src, dma_gather dst logits (EXTD1);
   ln(w) is folded into the logit so ACT expands exp(leaky(z)+lnw) into a
   packed bf16 scale tile and ONE packed-rate DVE multiply scales the
   payload; one one-hot (bf16) + one wide PSUM-accumulated matmul per
   128-edge tile aggregates messages and denominators (ex hi/lo columns).
   Per-block finalize is split: normalize+ELU at the accumulation stop,
   PE transpose/projection deferred 2 blocks to avoid PE head-of-line.
 - L1 finalize per block: normalize, +bias, ELU -> h; PE-transpose h and
   matmul with [Wmu|vmu|umu]/[Wlv|vlv|ulv] to get next-layer message rows
   xpmu/xplv + logits; rows go into an AllGather across the 8 cores.
 - All L2/3 dst-logit gathers are prefetched and compacted into resident
   SBUF during the AllGather (DMA/Pool/DVE otherwise idle there).
 - L2/3 edge phase: same staircase trick, H=1, mu and lv share one gather and
   one matmul per tile; same ACT-expanded exp scaling.
Outputs (mu, logvar) assembled host-side from per-core slices.
"""

import numpy as np

# ---- problem constants (hardcoded per contract) ----
N = 20000
E = 320000
FIN = 512
HID = 256
LAT = 128
H = 4
C1 = 64
NEG = 0.2
EPS = 1e-16

NC = 8
NOWN = 2500          # dst nodes per core
BLOCKS = 20          # 128-node blocks per core (2560 padded local nodes)
NLOC = BLOCKS * 128  # 2560
TPB = 18             # tiles (128 edges) per block (key(0) max block = 2174 edges;
                     # 18 gives ~5-sigma headroom for any same-distribution graph)
TPC = 8                    # tiles per gather chunk
CHUNK = TPC * 128          # 1024 idxs per dma_gather (hw limit ~1024)
NPADA = 160 * 128          # 20480 padded global rows (divisible by 512)
XW = 384                   # XPT row bf16 width (768B): xp 0:256, ss1 f32 @bytes 512:528
X2W = 384                  # XPT2 row: xpmu 0:128, xplv 128:256, ssmu/sslv f32 @bytes 512:520
EXW_ = 64                  # EXTD row f32 width (sd1 0:4, sdmu 4, sdlv 5)

_cache = {}


def _wrap_idxs(idx):
    n = idx.shape[0]
    t = np.zeros((128, n // 16), np.int16)
    w = idx.reshape(n // 16, 16).T.astype(np.int16)
    for g in range(8):
        t[g * 16:(g + 1) * 16, :] = w
    return t


def _colmajor(a):
    # per-edge array [EPAD] -> [128, TILES_PAD] tile-column layout
    return np.ascontiguousarray(a.reshape(-1, 128).T)


def _make_plan(edge_index):
    """Tile plan specialized to the actual graph: per 128-node block, the
    tile count is the max over cores of ceil(edges/128) (SPMD: one module
    for all cores). Cuts ~8% of padded edge slots vs uniform TPB."""
    dst = np.asarray(edge_index[1], np.int64)
    core = dst // NOWN
    loc = dst - core * NOWN
    blk = loc // 128
    cnt = np.bincount(core * BLOCKS + blk, minlength=NC * BLOCKS)
    cnt = cnt.reshape(NC, BLOCKS).max(axis=0)
    tpb = tuple(max(1, int(-(-c // 128))) for c in cnt)
    tiles = sum(tpb)
    tiles_pad = -(-tiles // TPC) * TPC
    boff = []
    o = 0
    for b in range(BLOCKS):
        boff.append(o)
        o += tpb[b]
    tb, tk = [], []
    for b in range(BLOCKS):
        for k in range(tpb[b]):
            tb.append(b)
            tk.append(k)
    return {"tpb": tpb, "boff": tuple(boff), "tb": tb, "tk": tk,
            "TILES": tiles, "TILES_PAD": tiles_pad,
            "NCHUNK": tiles_pad // TPC, "EPAD": tiles_pad * 128}


def _default_plan():
    class _EI:
        pass
    dst = np.repeat(np.arange(N, dtype=np.int64), 16)
    return _make_plan((None, dst))


def _build_module(plan=None, upto="full"):
    if plan is None:
        plan = _default_plan()
    TILES = plan["TILES"]
    TILES_PAD = plan["TILES_PAD"]
    NCHUNK = plan["NCHUNK"]
    EPAD = plan["EPAD"]
    tb, tk, tpb = plan["tb"], plan["tk"], plan["tpb"]
    import concourse.bacc as bacc
    import concourse.mybir as mybir
    import concourse.tile as tile

    f32 = mybir.dt.float32
    bf16 = mybir.dt.bfloat16
    i16 = mybir.dt.int16
    Alu = mybir.AluOpType
    Act = mybir.ActivationFunctionType

    nc = bacc.Bacc("TRN2", target_bir_lowering=False, num_devices=NC)

    # ---- inputs ----
    xTb = nc.dram_tensor("xTb", [FIN, NPADA], mybir.dt.bfloat16, kind="ExternalInput")
    xTown = nc.dram_tensor("xTown", [FIN, NLOC], f32, kind="ExternalInput")
    w1b = nc.dram_tensor("w1b", [FIN, 256], bf16, kind="ExternalInput")
    wsd_own = nc.dram_tensor("wsd_own", [FIN, 4], f32, kind="ExternalInput")
    att1repb = nc.dram_tensor("att1repb", [128, 256], mybir.dt.bfloat16, kind="ExternalInput")
    wmue = nc.dram_tensor("wmue", [HID, 130], bf16, kind="ExternalInput")  # [Wmu|vmu|umu]
    wlve = nc.dram_tensor("wlve", [HID, 130], bf16, kind="ExternalInput")
    b1b = nc.dram_tensor("b1b", [128, 256], f32, kind="ExternalInput")
    bmub = nc.dram_tensor("bmub", [128, 128], f32, kind="ExternalInput")
    blvb = nc.dram_tensor("blvb", [128, 128], f32, kind="ExternalInput")
    iota = nc.dram_tensor("iota", [128, 128], bf16, kind="ExternalInput")
    ident = nc.dram_tensor("ident", [128, 128], f32, kind="ExternalInput")
    srcg = nc.dram_tensor("srcg", [128, EPAD // 16], i16, kind="ExternalInput")
    src2 = nc.dram_tensor("src2", [128, EPAD // 16], i16, kind="ExternalInput")
    dstl = nc.dram_tensor("dstl", [128, EPAD // 16], i16, kind="ExternalInput")
    dstoffT = nc.dram_tensor("dstoffT", [128, TILES_PAD], f32, kind="ExternalInput")
    lnwT = nc.dram_tensor("lnwT", [128, TILES_PAD], f32, kind="ExternalInput")

    mulv_out = nc.dram_tensor("mulv_out", [NLOC, 2, LAT], f32,
                              kind="ExternalOutput")

    with tile.TileContext(nc) as tc:
        with (
            tc.tile_pool(name="cst", bufs=1) as cst,
            tc.tile_pool(name="lw", bufs=2) as lw,
            tc.tile_pool(name="xa", bufs=2) as xa,
            tc.tile_pool(name="gx", bufs=4) as gx,
            tc.tile_pool(name="ge", bufs=4) as ge,
            tc.tile_pool(name="ge2", bufs=4) as ge2,
            tc.tile_pool(name="xw", bufs=3) as xw,
            tc.tile_pool(name="oh", bufs=10) as ohp,
            tc.tile_pool(name="sm", bufs=6) as sm,
            tc.tile_pool(name="fin", bufs=3) as fin,
            tc.tile_pool(name="hbp", bufs=4) as hbp,
            tc.tile_pool(name="ps2", bufs=3, space="PSUM") as ps2,
            tc.tile_pool(name="ps1", bufs=1, space="PSUM") as ps1,
            tc.tile_pool(name="pst", bufs=2, space="PSUM") as pstp,
            tc.tile_pool(name="pmv", bufs=1, space="PSUM") as pmv,
            tc.tile_pool(name="dr", bufs=1, space="DRAM") as dr,
        ):
            # internal DRAM tables (pool tiles so Tile tracks RAW deps)
            XPT = dr.tile([NPADA, XW], bf16, tag="XPT")
            EXTD1 = dr.tile([NLOC, EXW_], f32, tag="EXTD1")
            EXTD2 = dr.tile([NLOC, EXW_], f32, tag="EXTD2")

            # resident compact dst-logit tables
            extd2c = cst.tile([128, NCHUNK, TPC, 2], f32, tag="extd2c")

            # resident constants
            w1b_t = []
            for kk in range(4):
                t = cst.tile([128, 256], bf16, tag=f"w1b{kk}")
                nc.sync.dma_start(t[:], w1b[kk * 128:(kk + 1) * 128, :])
                w1b_t.append(t)
            wsd_t = []
            for kk in range(4):
                t = cst.tile([128, 4], f32, tag=f"wsd{kk}")
                nc.sync.dma_start(t[:], wsd_own[kk * 128:(kk + 1) * 128, :])
                wsd_t.append(t)
            wmue_t = []
            wlve_t = []
            for kk in range(2):
                t = cst.tile([128, 130], bf16, tag=f"wmue{kk}")
                nc.sync.dma_start(t[:], wmue[kk * 128:(kk + 1) * 128, :])
                wmue_t.append(t)
                t2 = cst.tile([128, 130], bf16, tag=f"wlve{kk}")
                nc.sync.dma_start(t2[:], wlve[kk * 128:(kk + 1) * 128, :])
                wlve_t.append(t2)
            att1repb_t = cst.tile([128, 256], bf16, tag="att1repb")
            nc.sync.dma_start(att1repb_t[:], att1repb[:])
            b1b_t = cst.tile([128, 256], f32, tag="b1b")
            nc.sync.dma_start(b1b_t[:], b1b[:])
            bmub_t = cst.tile([128, 128], f32, tag="bmub")
            nc.sync.dma_start(bmub_t[:], bmub[:])
            blvb_t = cst.tile([128, 128], f32, tag="blvb")
            nc.sync.dma_start(blvb_t[:], blvb[:])
            iota_t = cst.tile([128, 128], bf16, tag="iota")
            nc.sync.dma_start(iota_t[:], iota[:])
            ident_t = cst.tile([128, 128], f32, tag="ident")
            nc.sync.dma_start(ident_t[:], ident[:])
            srcg_t = cst.tile([128, EPAD // 16], i16, tag="srcg")
            nc.sync.dma_start(srcg_t[:], srcg[:])
            src2_t = cst.tile([128, EPAD // 16], i16, tag="src2")
            nc.sync.dma_start(src2_t[:], src2[:])
            dstl_t = cst.tile([128, EPAD // 16], i16, tag="dstl")
            nc.sync.dma_start(dstl_t[:], dstl[:])
            dstoffT_t = cst.tile([128, TILES_PAD], f32, tag="dstoffT")
            nc.sync.dma_start(dstoffT_t[:], dstoffT[:])
            lnwT_t = cst.tile([128, TILES_PAD], f32, tag="lnwT")
            nc.sync.dma_start(lnwT_t[:], lnwT[:])

            # ---- phase A-own: sd1 for own nodes -> EXTD1[:,0:4] ----
            # 4 blocks per DMA: HWDGE descriptor-gen is ~625ns per dma_start,
            # so batch transfers
            for b4 in range(BLOCKS // 4):
                lx = lw.tile([128, 4, 4, 128], f32, tag="lxo")
                nc.scalar.dma_start(
                    lx[:], xTown[:].rearrange("(kk p) (b j) -> p kk b j",
                                              p=128, j=128)
                    [:, :, 4 * b4:4 * b4 + 4, :])
                sds4 = xa.tile([128, 4, 4], f32, tag="sds")
                for bb in range(4):
                    ps = ps1.tile([128, 8], f32, tag="pslog", name="psOwn")
                    for kk in range(4):
                        nc.tensor.matmul(ps[:, 0:4], lx[:, kk, bb, :],
                                         wsd_t[kk][:],
                                         start=(kk == 0), stop=(kk == 3))
                    nc.scalar.copy(sds4[:, bb, :], ps[:, 0:4])
                nc.sync.dma_start(
                    EXTD1[b4 * 512:(b4 + 1) * 512, 0:4]
                    .rearrange("(bb p) c -> p bb c", p=128), sds4[:])

            # ---- phase A: XPT[n] = [xp bf16 (256) | ss1 f32 (4)] ----
            # bf16 x loads only; ss1 = (xp_psum * att_src_rep) grouped-reduce
            # on DVE straight from PSUM (f32 accumulation, no f32 x needed)
            for g2 in range(NPADA // 1024):
                lxb = lw.tile([128, 4, 1024], bf16, tag="lxb")
                nc.scalar.dma_start(
                    lxb[:], xTb[:].rearrange("(kk p) (g j) -> p kk g j",
                                             p=128, j=1024)[:, :, g2, :])
                xps = xa.tile([128, 8, 264], bf16, tag="xps")
                for ti in range(8):
                    ps = ps2.tile([128, 256], f32, tag="blk", name="psA")
                    for kk in range(4):
                        sl = slice(ti * 128, (ti + 1) * 128)
                        nc.tensor.matmul(ps[:], lxb[:, kk, sl], w1b_t[kk][:],
                                         start=(kk == 0), stop=(kk == 3))
                    sstmp = xa.tile([128, 256], bf16, tag="sstmp")
                    nc.vector.tensor_tensor(sstmp[:], ps[:], att1repb_t[:],
                                            op=Alu.mult)
                    nc.vector.tensor_reduce(
                        xps[:, ti, 256:264].bitcast(f32),
                        sstmp[:].rearrange("p (h c) -> p h c", c=64),
                        axis=mybir.AxisListType.X, op=Alu.add)
                    nc.scalar.copy(xps[:, ti, 0:256], ps[:])
                nc.sync.dma_start(
                    XPT[:].rearrange("(g8 p) c -> p g8 c", p=128)
                    [:, 8 * g2:8 * g2 + 8, 0:264], xps[:])

            # AllGather buffers for layer-2/3 message rows
            agin = dr.tile([NLOC, X2W], bf16, tag="agin")
            agout = dr.tile([NC * NLOC, X2W], bf16, tag="agout",
                            addr_space="Shared")

            # ---- L1 edge phase + finalize ----
            def _l1_stage2(b, hb):
                # transpose h (2 x 128x128), cast to bf16, project
                hTs = []
                for half in range(2):
                    pst = pstp.tile([128, 128], f32, tag="pst")
                    nc.tensor.transpose(
                        pst[:], hb[:, half * 128:(half + 1) * 128],
                        ident_t[:])
                    hT = fin.tile([128, 128], bf16, tag=f"hT{half}")
                    nc.vector.tensor_copy(hT[:], pst[:])
                    hTs.append(hT)
                psmu = pmv.tile([128, 130], f32, tag="psmu")
                pslv = pmv.tile([128, 130], f32, tag="pslv")
                for kk in range(2):
                    nc.tensor.matmul(psmu[:], hTs[kk][:], wmue_t[kk][:],
                                     start=(kk == 0), stop=(kk == 1))
                    nc.tensor.matmul(pslv[:], hTs[kk][:], wlve_t[kk][:],
                                     start=(kk == 0), stop=(kk == 1))
                xr2 = fin.tile([128, 260], bf16, tag="xr2")
                nc.scalar.copy(xr2[:, 0:128], psmu[:, 0:128])
                nc.scalar.copy(xr2[:, 128:256], pslv[:, 0:128])
                ssv = xr2[:, 256:260].bitcast(f32)
                nc.vector.tensor_copy(ssv[:, 0:1], psmu[:, 128:129])
                nc.vector.tensor_copy(ssv[:, 1:2], pslv[:, 128:129])
                nc.sync.dma_start(
                    agin[b * 128:(b + 1) * 128, 0:260], xr2[:])
                sd2 = sm.tile([128, 2], f32, tag="sd2")
                nc.vector.tensor_copy(sd2[:, 0:1], psmu[:, 129:130])
                nc.vector.tensor_copy(sd2[:, 1:2], pslv[:, 129:130])
                nc.sync.dma_start(
                    EXTD2[b * 128:(b + 1) * 128, 0:2], sd2[:])

            pend1 = {}
            blk_ps = {}
            for ci in range(NCHUNK if upto != "A" else 0):
                xrow = gx.tile([128, TPC, XW], bf16, tag="xrow")
                nc.gpsimd.dma_gather(
                    xrow[:], XPT[:], srcg_t[:, ci * 64:(ci + 1) * 64],
                    CHUNK, CHUNK, XW)
                extd = ge.tile([128, TPC, EXW_], f32, tag="extd")
                nc.gpsimd.dma_gather(
                    extd[:], EXTD1[:], dstl_t[:, ci * 64:(ci + 1) * 64],
                    CHUNK, CHUNK, EXW_)
                # alpha for the whole chunk: z=ss+sd, leaky
                z = sm.tile([128, TPC, 4], f32, tag="z")
                nc.vector.tensor_tensor(z[:], xrow[:, :, 256:264].bitcast(f32),
                                        extd[:, :, 0:4], op=Alu.add)
                nc.vector.scalar_tensor_tensor(
                    z[:], in0=z[:], scalar=NEG, in1=z[:],
                    op0=Alu.mult, op1=Alu.max)
                # unweighted exp for denominators (f32)
                ex = sm.tile([128, TPC, 4], f32, tag="ex")
                nc.scalar.activation(ex[:], z[:], Act.Exp)
                # zw = leaky(z) + ln(w); exp via ACT expanded to a packed
                # per-column scale tile, then ONE packed DVE multiply (the
                # stride-0 broadcast form runs at half DVE rate in hardware)
                zw = sm.tile([128, TPC, 4], f32, tag="zw")
                wb = lnwT_t[:, ci * TPC:(ci + 1) * TPC]
                nc.vector.tensor_tensor(
                    zw[:], z[:],
                    wb.rearrange("p (t o) -> p t o", o=1).to_broadcast(
                        [128, TPC, 4]), op=Alu.add)
                exwb = xw.tile([128, TPC, 4, 64], bf16, tag="exwb")
                nc.scalar.activation(
                    exwb[:],
                    zw[:].rearrange("p t (h o) -> p t h o", o=1)
                    .to_broadcast([128, TPC, 4, 64]), Act.Exp)
                nc.vector.tensor_tensor(
                    xrow[:, :, 0:256], xrow[:, :, 0:256],
                    exwb[:].rearrange("p t h c -> p t (h c)"), op=Alu.mult)
                # unweighted ex -> hi/lo bf16 denominator cols 256:264
                nc.vector.tensor_copy(xrow[:, :, 256:260], ex[:])
                nc.vector.tensor_tensor(xrow[:, :, 260:264], ex[:],
                                        xrow[:, :, 256:260], op=Alu.subtract)

                for tt in range(TPC):
                    t = ci * TPC + tt
                    if t >= TILES:
                        continue
                    b = tb[t]
                    k = tk[t]
                    if k == 0:
                        # flush deferred stage-2 of blocks done >= 2 ago so
                        # their PE work never head-of-line-blocks these matmuls
                        for bb in sorted(pend1):
                            if bb <= b - 2:
                                _l1_stage2(bb, pend1.pop(bb))
                        blk_ps[b] = ps2.tile([128, 264], f32, tag="blk",
                                             name="blkps")
                    if b == BLOCKS - 1 and k == 4 and (b - 1) in pend1:
                        # tail: drain the second-to-last block's stage-2 while
                        # the last block is still accumulating
                        _l1_stage2(b - 1, pend1.pop(b - 1))
                    ps = blk_ps[b]
                    ohx = ohp.tile([128, 128], bf16, tag="ohx")
                    nc.vector.tensor_scalar(
                        ohx[:], iota_t[:], dstoffT_t[:, t:t + 1], None,
                        Alu.is_equal)
                    nc.tensor.matmul(
                        ps[:, 0:264], ohx[:], xrow[:, tt, 0:264],
                        start=(k == 0), stop=(k == tpb[b] - 1))
                    if k == tpb[b] - 1:
                        # stage-1: normalize + ELU -> hb (DVE/ACT only)
                        den8 = sm.tile([128, 8], f32, tag="den8")
                        nc.vector.tensor_copy(den8[:], ps[:, 256:264])
                        den = sm.tile([128, 4], f32, tag="den")
                        nc.vector.tensor_tensor(den[:], den8[:, 0:4],
                                                den8[:, 4:8], op=Alu.add)
                        nc.vector.tensor_scalar_add(den[:], den[:], EPS)
                        rec = sm.tile([128, 4], f32, tag="rec")
                        nc.vector.reciprocal(rec[:], den[:])
                        hb = hbp.tile([128, 256], f32, tag="hb")
                        for h in range(H):
                            nc.vector.scalar_tensor_tensor(
                                hb[:, h * 64:(h + 1) * 64],
                                in0=ps[:, h * 64:(h + 1) * 64],
                                scalar=rec[:, h:h + 1],
                                in1=b1b_t[:, h * 64:(h + 1) * 64],
                                op0=Alu.mult, op1=Alu.add)
                        # ELU: h = max(z,0) + exp(min(z,0)) - 1
                        zm = fin.tile([128, 256], f32, tag="zm")
                        nc.vector.tensor_scalar_min(zm[:], hb[:], 0.0)
                        ez = fin.tile([128, 256], f32, tag="ez")
                        nc.scalar.activation(ez[:], zm[:], Act.Exp)
                        hr = fin.tile([128, 256], f32, tag="hr")
                        nc.scalar.activation(hr[:], hb[:], Act.Relu)
                        nc.vector.scalar_tensor_tensor(
                            hb[:], in0=ez[:], scalar=-1.0, in1=hr[:],
                            op0=Alu.add, op1=Alu.add)
                        pend1[b] = hb
                        del blk_ps[b]
            for bb in sorted(pend1):
                _l1_stage2(bb, pend1.pop(bb))

            # ---- exchange layer-2/3 message rows ----
            if upto in ("AG", "full"):
                nc.gpsimd.collective_compute(
                    "AllGather", mybir.AluOpType.bypass,
                    replica_groups=[list(range(NC))],
                    ins=[agin.opt()], outs=[agout.opt()])

            # pre-gather ALL L2/3 dst logits; their DMA + the compaction
            # run under the AllGather, when every engine is otherwise idle
            if upto == "full":
                for ci in range(NCHUNK):
                    ep = ge2.tile([128, TPC, EXW_], f32, tag="extd_pre",
                                  name="extdpre", bufs=4)
                    nc.gpsimd.dma_gather(
                        ep[:], EXTD2[:], dstl_t[:, ci * 64:(ci + 1) * 64],
                        CHUNK, CHUNK, EXW_)
                    # compact sd2 into the resident tile; full staging tile
                    # recycles while the AllGather keeps the DMA engines idle
                    nc.vector.tensor_copy(extd2c[:, ci, :, :],
                                          ep[:, :, 0:2])

            # ---- L2/3 edge phase (mu and lv share gathers) ----
            blk2 = {}
            for ci in range(NCHUNK if upto == "full" else 0):
                xrow = gx.tile([128, TPC, X2W], bf16, tag="xrow2")
                nc.gpsimd.dma_gather(
                    xrow[:], agout[:], src2_t[:, ci * 64:(ci + 1) * 64],
                    CHUNK, CHUNK, X2W)
                z = sm.tile([128, TPC, 2], f32, tag="z2")
                nc.vector.tensor_tensor(z[:], xrow[:, :, 256:260].bitcast(f32),
                                        extd2c[:, ci, :, :], op=Alu.add)
                nc.vector.scalar_tensor_tensor(
                    z[:], in0=z[:], scalar=NEG, in1=z[:],
                    op0=Alu.mult, op1=Alu.max)
                ex = sm.tile([128, TPC, 2], f32, tag="ex2")
                nc.scalar.activation(ex[:], z[:], Act.Exp)
                zw = sm.tile([128, TPC, 2], f32, tag="zw2")
                wb = lnwT_t[:, ci * TPC:(ci + 1) * TPC]
                nc.vector.tensor_tensor(
                    zw[:], z[:],
                    wb.rearrange("p (t o) -> p t o", o=1).to_broadcast(
                        [128, TPC, 2]), op=Alu.add)
                exwb = xw.tile([128, TPC, 2, 128], bf16, tag="exwb")
                nc.scalar.activation(
                    exwb[:],
                    zw[:].rearrange("p t (l o) -> p t l o", o=1)
                    .to_broadcast([128, TPC, 2, 128]), Act.Exp)
                nc.vector.tensor_tensor(
                    xrow[:, :, 0:256], xrow[:, :, 0:256],
                    exwb[:].rearrange("p t l c -> p t (l c)"), op=Alu.mult)
                nc.vector.tensor_copy(xrow[:, :, 256:258], ex[:])
                nc.vector.tensor_tensor(xrow[:, :, 258:260], ex[:],
                                        xrow[:, :, 256:258], op=Alu.subtract)

                for tt in range(TPC):
                    t = ci * TPC + tt
                    if t >= TILES:
                        continue
                    b = tb[t]
                    k = tk[t]
                    if k == 0:
                        blk2[b] = ps2.tile([128, 260], f32, tag="blk",
                                           name="blk2ps")
                    ps2t = blk2[b]
                    ohx = ohp.tile([128, 128], bf16, tag="ohx")
                    nc.vector.tensor_scalar(
                        ohx[:], iota_t[:], dstoffT_t[:, t:t + 1], None,
                        Alu.is_equal)
                    nc.tensor.matmul(
                        ps2t[:, 0:260], ohx[:], xrow[:, tt, 0:260],
                        start=(k == 0), stop=(k == tpb[b] - 1))
                    if k == tpb[b] - 1:
                        den4 = sm.tile([128, 4], f32, tag="den4")
                        nc.vector.tensor_copy(den4[:], ps2t[:, 256:260])
                        den = sm.tile([128, 2], f32, tag="den2")
                        nc.vector.tensor_tensor(
                            den[:], den4[:, 0:2], den4[:, 2:4], op=Alu.add)
                        nc.vector.tensor_scalar_add(den[:], den[:], EPS)
                        rec = sm.tile([128, 2], f32, tag="rec2")
                        nc.vector.reciprocal(rec[:], den[:])
                        ob = fin.tile([128, 2, 128], f32, tag="ob", bufs=4)
                        for li, bias_t in enumerate((bmub_t, blvb_t)):
                            nc.vector.scalar_tensor_tensor(
                                ob[:, li, :],
                                in0=ps2t[:, li * 128:(li + 1) * 128],
                                scalar=rec[:, li:li + 1],
                                in1=bias_t[:], op0=Alu.mult, op1=Alu.add)
                        nc.sync.dma_start(
                            mulv_out[b * 128:(b + 1) * 128, :, :], ob[:])
                        del blk2[b]

    nc.compile()
    return nc


def _prep_inputs(plan, x, edge_index, edge_weight, W1, att1, b1, Wmu, attmu,
                 bmu, Wlv, attlv, blv):
    EPAD = plan["EPAD"]
    boff, tpb = plan["boff"], plan["tpb"]
    import ml_dtypes
    bf = ml_dtypes.bfloat16

    src = np.asarray(edge_index[0], np.int64)
    dst = np.asarray(edge_index[1], np.int64)
    w = np.asarray(edge_weight, np.float32)
    x = np.asarray(x, np.float32)

    # fused weights
    att1 = np.asarray(att1, np.float32)          # [H, 2*C1]
    W1 = np.asarray(W1, np.float32)
    Wsd1 = np.zeros((FIN, H), np.float32)
    for h in range(H):
        Wsd1[:, h] = W1[:, h * C1:(h + 1) * C1] @ att1[h, :C1]
    # replicated src-attention row for on-chip ss1 reduce
    att1repb = np.tile(att1[:, C1:].reshape(1, H * C1), (128, 1)).astype(bf)

    attmu = np.asarray(attmu, np.float32).reshape(-1)        # [2*LAT]
    attlv = np.asarray(attlv, np.float32).reshape(-1)
    Wmu = np.asarray(Wmu, np.float32)
    Wlv = np.asarray(Wlv, np.float32)
    wmue = np.concatenate(
        [Wmu, (Wmu @ attmu[LAT:])[:, None], (Wmu @ attmu[:LAT])[:, None]],
        axis=1).astype(bf)
    wlve = np.concatenate(
        [Wlv, (Wlv @ attlv[LAT:])[:, None], (Wlv @ attlv[:LAT])[:, None]],
        axis=1).astype(bf)

    xT = np.zeros((FIN, NPADA), np.float32)
    xT[:, :N] = x.T
    xTb = xT.astype(bf)
    b1b = np.tile(np.asarray(b1, np.float32)[None, :], (128, 1))
    bmub = np.tile(np.asarray(bmu, np.float32)[None, :], (128, 1))
    blvb = np.tile(np.asarray(blv, np.float32)[None, :], (128, 1))
    iota = np.tile(np.arange(128, dtype=np.float32)[None, :],
                   (128, 1)).astype(bf)
    ident = np.eye(128, dtype=np.float32)

    # sort edges by dst, bucket per core, pad per 128-node block to TPB tiles
    order = np.argsort(dst, kind="stable")
    ssrc, sdst, sw = src[order], dst[order], w[order]
    core_of = sdst // NOWN
    in_maps = []
    for c in range(NC):
        m = core_of == c
        cs, cd, cw = ssrc[m], sdst[m] - c * NOWN, sw[m]
        blk = cd // 128
        e_src = np.zeros(EPAD, np.int64)
        e_dstloc = np.zeros(EPAD, np.int64)
        e_dstoff = np.full(EPAD, -1.0, np.float32)
        e_w = np.zeros(EPAD, np.float32)
        for b in range(BLOCKS):
            bm = blk == b
            nbe = int(bm.sum())
            if nbe > tpb[b] * 128:
                raise RuntimeError(f"block overflow core {c} block {b}: {nbe}")
            o = boff[b] * 128
            e_src[o:o + nbe] = cs[bm]
            e_dstloc[o:o + nbe] = cd[bm]
            e_dstoff[o:o + nbe] = (cd[bm] - b * 128).astype(np.float32)
            e_w[o:o + nbe] = cw[bm]
        e_lnw = np.full(EPAD, -60.0, np.float32)
        real = e_dstoff >= 0
        e_lnw[real] = np.log(np.maximum(e_w[real], 1e-38))
        own = e_src // NOWN
        e_src2 = own * NLOC + (e_src - own * NOWN)
        xTown = np.zeros((FIN, NLOC), np.float32)
        xTown[:, :NOWN] = x.T[:, c * NOWN:(c + 1) * NOWN]
        in_maps.append({
            "xTb": xTb, "xTown": xTown, "w1b": W1.astype(bf),
            "att1repb": att1repb,
            "wsd_own": Wsd1, "wmue": wmue, "wlve": wlve, "b1b": b1b,
            "bmub": bmub, "blvb": blvb, "iota": iota, "ident": ident,
            "srcg": _wrap_idxs(e_src), "src2": _wrap_idxs(e_src2),
            "dstl": _wrap_idxs(e_dstloc),
            "dstoffT": _colmajor(e_dstoff),
            "lnwT": _colmajor(e_lnw),
        })
    return in_maps


def kernel(x, edge_index, edge_weight, W1, att1, b1, Wmu, attmu, bmu,
           Wlv, attlv, blv):
    from concourse.bass_utils import run_bass_kernel_spmd

    plan = _make_plan(edge_index)
    key = plan["tpb"]
    if key not in _cache:
        _cache[key] = _build_module(plan)
    nc = _cache[key]
    _cache["nc"] = nc
    in_maps = _prep_inputs(plan, x, edge_index, edge_weight, W1, att1, b1,
                           Wmu, attmu, bmu, Wlv, attlv, blv)
    r = run_bass_kernel_spmd(nc, in_maps, list(range(NC)))
    mu = np.zeros((N, LAT), np.float32)
    lv = np.zeros((N, LAT), np.float32)
    for c in range(NC):
        out = r.results[c]["mulv_out"]
        mu[c * NOWN:(c + 1) * NOWN] = out[:NOWN, 0]
        lv[c * NOWN:(c + 1) * NOWN] = out[:NOWN, 1]
    return (mu, lv)



# revision 29
# speedup vs baseline: 1.0311x; 1.0311x over previous
"""GAT encoder (3 GAT layers: 256-hid 4-head concat + mu/logvar 128) on 8 trn2 cores.

Strategy (dst-range node sharding, per sharding_hint):
 - Host sorts edges by dst, buckets per core (2500 dst nodes each), pads each
   128-node block to TPB tiles of 128 edges.
 - Tile plan is JIT-specialized to the actual graph (per-block tile count =
   max over cores of ceil(edges/128)).
 - Phase A (per core, redundant): xp = x @ W1 (bf16 matmul, batched bf16-only
   loads) for all nodes; ss1 logits reduced on-chip from the f32 PSUM
   (mult by replicated att row + grouped tensor_reduce); rows -> DRAM XPT
   (bf16 payload + f32 logit bytes). sd1 for own nodes via f32 matmul ->
   EXTD1 (separate from EXTD2 so L1 gathers never alias finalize writes).
 - L1 edge phase: dma_gather xp rows by src, dma_gather dst logits (EXTD1);
   ln(w) is folded into the logit so ACT expands exp(leaky(z)+lnw) into a
   packed bf16 scale tile and ONE packed-rate DVE multiply scales the
   payload; one one-hot (bf16) + one wide PSUM-accumulated matmul per
   128-edge tile aggregates messages and denominators (ex hi/lo columns).
   Per-block finalize is split: normalize+ELU at the accumulation stop,
   PE transpose/projection deferred 2 blocks to avoid PE head-of-line.
 - L1 finalize per block: normalize, +bias, ELU -> h; PE-transpose h and
   matmul with [Wmu|vmu|umu]/[Wlv|vlv|ulv] to get next-layer message rows
   xpmu/xplv + logits; rows go into an AllGather across the 8 cores.
 - All L2/3 dst-logit gathers are prefetched and compacted into resident
   SBUF during the AllGather (DMA/Pool/DVE otherwise idle there).
 - L2/3 edge phase: same staircase trick, H=1, mu and lv share one gather and
   one matmul per tile; same ACT-expanded exp scaling.
Outputs (mu, logvar) assembled host-side from per-core slices.
"""

import numpy as np

# ---- problem constants (hardcoded per contract) ----
N = 20000
E = 320000
FIN = 512
HID = 256
LAT = 128
H = 4
C1 = 64
NEG = 0.2
EPS = 1e-16

NC = 8
NOWN = 2500          # dst nodes per core
BLOCKS = 20          # 128-node blocks per core (2560 padded local nodes)
NLOC = BLOCKS * 128  # 2560
TPB = 18             # tiles (128 edges) per block (key(0) max block = 2174 edges;
                     # 18 gives ~5-sigma headroom for any same-distribution graph)
TPC = 8                    # tiles per gather chunk
CHUNK = TPC * 128          # 1024 idxs per dma_gather (hw limit ~1024)
NPADA = 160 * 128          # 20480 padded global rows (divisible by 512)
XW = 384                   # XPT row bf16 width (768B): xp 0:256, ss1 f32 @bytes 512:528
X2W = 384                  # XPT2 row: xpmu 0:128, xplv 128:256, ssmu/sslv f32 @bytes 512:520
EXW_ = 64                  # EXTD row f32 width (sd1 0:4, sdmu 4, sdlv 5)

_cache = {}


def _wrap_idxs(idx):
    n = idx.shape[0]
    t = np.zeros((128, n // 16), np.int16)
    w = idx.reshape(n // 16, 16).T.astype(np.int16)
    for g in range(8):
        t[g * 16:(g + 1) * 16, :] = w
    return t


def _colmajor(a):
    # per-edge array [EPAD] -> [128, TILES_PAD] tile-column layout
    return np.ascontiguousarray(a.reshape(-1, 128).T)


def _make_plan(edge_index):
    """Tile plan specialized to the actual graph: per 128-node block, the
    tile count is the max over cores of ceil(edges/128) (SPMD: one module
    for all cores). Cuts ~8% of padded edge slots vs uniform TPB."""
    dst = np.asarray(edge_index[1], np.int64)
    core = dst // NOWN
    loc = dst - core * NOWN
    blk = loc // 128
    cnt = np.bincount(core * BLOCKS + blk, minlength=NC * BLOCKS)
    cnt = cnt.reshape(NC, BLOCKS).max(axis=0)
    tpb = tuple(max(1, int(-(-c // 128))) for c in cnt)
    tiles = sum(tpb)
    tiles_pad = -(-tiles // TPC) * TPC
    boff = []
    o = 0
    for b in range(BLOCKS):
        boff.append(o)
        o += tpb[b]
    tb, tk = [], []
    for b in range(BLOCKS):
        for k in range(tpb[b]):
            tb.append(b)
            tk.append(k)
    return {"tpb": tpb, "boff": tuple(boff), "tb": tb, "tk": tk,
            "TILES": tiles, "TILES_PAD": tiles_pad,
            "NCHUNK": tiles_pad // TPC, "EPAD": tiles_pad * 128}


def _default_plan():
    class _EI:
        pass
    dst = np.repeat(np.arange(N, dtype=np.int64), 16)
    return _make_plan((None, dst))


def _build_module(plan=None, upto="full"):
    if plan is None:
        plan = _default_plan()
    TILES = plan["TILES"]
    TILES_PAD = plan["TILES_PAD"]
    NCHUNK = plan["NCHUNK"]
    EPAD = plan["EPAD"]
    tb, tk, tpb = plan["tb"], plan["tk"], plan["tpb"]
    import concourse.bacc as bacc
    import concourse.mybir as mybir
    import concourse.tile as tile

    f32 = mybir.dt.float32
    bf16 = mybir.dt.bfloat16
    i16 = mybir.dt.int16
    Alu = mybir.AluOpType
    Act = mybir.ActivationFunctionType

    nc = bacc.Bacc("TRN2", target_bir_lowering=False, num_devices=NC)

    # ---- inputs ----
    xTb = nc.dram_tensor("xTb", [FIN, NPADA], mybir.dt.bfloat16, kind="ExternalInput")
    xTown = nc.dram_tensor("xTown", [FIN, NLOC], f32, kind="ExternalInput")
    w1b = nc.dram_tensor("w1b", [FIN, 256], bf16, kind="ExternalInput")
    wsd_own = nc.dram_tensor("wsd_own", [FIN, 4], f32, kind="ExternalInput")
    att1repb = nc.dram_tensor("att1repb", [128, 256], mybir.dt.bfloat16, kind="ExternalInput")
    wmue = nc.dram_tensor("wmue", [HID, 130], bf16, kind="ExternalInput")  # [Wmu|vmu|umu]
    wlve = nc.dram_tensor("wlve", [HID, 130], bf16, kind="ExternalInput")
    b1b = nc.dram_tensor("b1b", [128, 256], f32, kind="ExternalInput")
    bmub = nc.dram_tensor("bmub", [128, 128], f32, kind="ExternalInput")
    blvb = nc.dram_tensor("blvb", [128, 128], f32, kind="ExternalInput")
    iota = nc.dram_tensor("iota", [128, 128], bf16, kind="ExternalInput")
    ident = nc.dram_tensor("ident", [128, 128], f32, kind="ExternalInput")
    srcg = nc.dram_tensor("srcg", [128, EPAD // 16], i16, kind="ExternalInput")
    src2 = nc.dram_tensor("src2", [128, EPAD // 16], i16, kind="ExternalInput")
    dstl = nc.dram_tensor("dstl", [128, EPAD // 16], i16, kind="ExternalInput")
    dstoffT = nc.dram_tensor("dstoffT", [128, TILES_PAD], f32, kind="ExternalInput")
    lnwT = nc.dram_tensor("lnwT", [128, TILES_PAD], f32, kind="ExternalInput")

    mulv_out = nc.dram_tensor("mulv_out", [NLOC, 2, LAT], f32,
                              kind="ExternalOutput")

    with tile.TileContext(nc) as tc:
        with (
            tc.tile_pool(name="cst", bufs=1) as cst,
            tc.tile_pool(name="lw", bufs=2) as lw,
            tc.tile_pool(name="xa", bufs=2) as xa,
            tc.tile_pool(name="gx", bufs=5) as gx,
            tc.tile_pool(name="ge", bufs=4) as ge,
            tc.tile_pool(name="ge2", bufs=3) as ge2,
            tc.tile_pool(name="xw", bufs=3) as xw,
            tc.tile_pool(name="oh", bufs=10) as ohp,
            tc.tile_pool(name="sm", bufs=6) as sm,
            tc.tile_pool(name="fin", bufs=3) as fin,
            tc.tile_pool(name="hbp", bufs=4) as hbp,
            tc.tile_pool(name="ps2", bufs=3, space="PSUM") as ps2,
            tc.tile_pool(name="ps1", bufs=1, space="PSUM") as ps1,
            tc.tile_pool(name="pst", bufs=2, space="PSUM") as pstp,
            tc.tile_pool(name="pmv", bufs=1, space="PSUM") as pmv,
            tc.tile_pool(name="dr", bufs=1, space="DRAM") as dr,
        ):
            # internal DRAM tables (pool tiles so Tile tracks RAW deps)
            XPT = dr.tile([NPADA, XW], bf16, tag="XPT")
            EXTD1 = dr.tile([NLOC, EXW_], f32, tag="EXTD1")
            EXTD2 = dr.tile([NLOC, EXW_], f32, tag="EXTD2")

            # resident compact dst-logit tables
            extd2c = cst.tile([128, NCHUNK, TPC, 2], f32, tag="extd2c")

            # resident constants
            w1b_t = []
            for kk in range(4):
                t = cst.tile([128, 256], bf16, tag=f"w1b{kk}")
                nc.sync.dma_start(t[:], w1b[kk * 128:(kk + 1) * 128, :])
                w1b_t.append(t)
            wsd_t = []
            for kk in range(4):
                t = cst.tile([128, 4], f32, tag=f"wsd{kk}")
                nc.sync.dma_start(t[:], wsd_own[kk * 128:(kk + 1) * 128, :])
                wsd_t.append(t)
            wmue_t = []
            wlve_t = []
            for kk in range(2):
                t = cst.tile([128, 130], bf16, tag=f"wmue{kk}")
                nc.sync.dma_start(t[:], wmue[kk * 128:(kk + 1) * 128, :])
                wmue_t.append(t)
                t2 = cst.tile([128, 130], bf16, tag=f"wlve{kk}")
                nc.sync.dma_start(t2[:], wlve[kk * 128:(kk + 1) * 128, :])
                wlve_t.append(t2)
            att1repb_t = cst.tile([128, 256], bf16, tag="att1repb")
            nc.sync.dma_start(att1repb_t[:], att1repb[:])
            b1b_t = cst.tile([128, 256], f32, tag="b1b")
            nc.sync.dma_start(b1b_t[:], b1b[:])
            bmub_t = cst.tile([128, 128], f32, tag="bmub")
            nc.sync.dma_start(bmub_t[:], bmub[:])
            blvb_t = cst.tile([128, 128], f32, tag="blvb")
            nc.sync.dma_start(blvb_t[:], blvb[:])
            iota_t = cst.tile([128, 128], bf16, tag="iota")
            nc.sync.dma_start(iota_t[:], iota[:])
            ident_t = cst.tile([128, 128], f32, tag="ident")
            nc.sync.dma_start(ident_t[:], ident[:])
            srcg_t = cst.tile([128, EPAD // 16], i16, tag="srcg")
            nc.sync.dma_start(srcg_t[:], srcg[:])
            src2_t = cst.tile([128, EPAD // 16], i16, tag="src2")
            nc.sync.dma_start(src2_t[:], src2[:])
            dstl_t = cst.tile([128, EPAD // 16], i16, tag="dstl")
            nc.sync.dma_start(dstl_t[:], dstl[:])
            dstoffT_t = cst.tile([128, TILES_PAD], f32, tag="dstoffT")
            nc.sync.dma_start(dstoffT_t[:], dstoffT[:])
            lnwT_t = cst.tile([128, TILES_PAD], f32, tag="lnwT")
            nc.sync.dma_start(lnwT_t[:], lnwT[:])

            # ---- phase A-own: sd1 for own nodes -> EXTD1[:,0:4] ----
            # 4 blocks per DMA: HWDGE descriptor-gen is ~625ns per dma_start,
            # so batch transfers
            for b4 in range(BLOCKS // 4):
                lx = lw.tile([128, 4, 4, 128], f32, tag="lxo")
                nc.scalar.dma_start(
                    lx[:], xTown[:].rearrange("(kk p) (b j) -> p kk b j",
                                              p=128, j=128)
                    [:, :, 4 * b4:4 * b4 + 4, :])
                sds4 = xa.tile([128, 4, 4], f32, tag="sds")
                for bb in range(4):
                    ps = ps1.tile([128, 8], f32, tag="pslog", name="psOwn")
                    for kk in range(4):
                        nc.tensor.matmul(ps[:, 0:4], lx[:, kk, bb, :],
                                         wsd_t[kk][:],
                                         start=(kk == 0), stop=(kk == 3))
                    nc.scalar.copy(sds4[:, bb, :], ps[:, 0:4])
                nc.sync.dma_start(
                    EXTD1[b4 * 512:(b4 + 1) * 512, 0:4]
                    .rearrange("(bb p) c -> p bb c", p=128), sds4[:])

            # ---- phase A: XPT[n] = [xp bf16 (256) | ss1 f32 (4)] ----
            # bf16 x loads only; ss1 = (xp_psum * att_src_rep) grouped-reduce
            # on DVE straight from PSUM (f32 accumulation, no f32 x needed)
            for g2 in range(NPADA // 1024):
                lxb = lw.tile([128, 4, 1024], bf16, tag="lxb")
                nc.scalar.dma_start(
                    lxb[:], xTb[:].rearrange("(kk p) (g j) -> p kk g j",
                                             p=128, j=1024)[:, :, g2, :])
                xps = xa.tile([128, 8, 264], bf16, tag="xps")
                for ti in range(8):
                    ps = ps2.tile([128, 256], f32, tag="blk", name="psA")
                    for kk in range(4):
                        sl = slice(ti * 128, (ti + 1) * 128)
                        nc.tensor.matmul(ps[:], lxb[:, kk, sl], w1b_t[kk][:],
                                         start=(kk == 0), stop=(kk == 3))
                    sstmp = xa.tile([128, 256], bf16, tag="sstmp")
                    nc.vector.tensor_tensor(sstmp[:], ps[:], att1repb_t[:],
                                            op=Alu.mult)
                    nc.vector.tensor_reduce(
                        xps[:, ti, 256:264].bitcast(f32),
                        sstmp[:].rearrange("p (h c) -> p h c", c=64),
                        axis=mybir.AxisListType.X, op=Alu.add)
                    nc.scalar.copy(xps[:, ti, 0:256], ps[:])
                nc.sync.dma_start(
                    XPT[:].rearrange("(g8 p) c -> p g8 c", p=128)
                    [:, 8 * g2:8 * g2 + 8, 0:264], xps[:])

            # AllGather buffers for layer-2/3 message rows
            agin = dr.tile([NLOC, X2W], bf16, tag="agin")
            agout = dr.tile([NC * NLOC, X2W], bf16, tag="agout",
                            addr_space="Shared")

            # ---- L1 edge phase + finalize ----
            def _l1_stage2(b, hb):
                # transpose h (2 x 128x128), cast to bf16, project
                hTs = []
                for half in range(2):
                    pst = pstp.tile([128, 128], f32, tag="pst")
                    nc.tensor.transpose(
                        pst[:], hb[:, half * 128:(half + 1) * 128],
                        ident_t[:])
                    hT = fin.tile([128, 128], bf16, tag=f"hT{half}")
                    nc.vector.tensor_copy(hT[:], pst[:])
                    hTs.append(hT)
                psmu = pmv.tile([128, 130], f32, tag="psmu")
                pslv = pmv.tile([128, 130], f32, tag="pslv")
                for kk in range(2):
                    nc.tensor.matmul(psmu[:], hTs[kk][:], wmue_t[kk][:],
                                     start=(kk == 0), stop=(kk == 1))
                    nc.tensor.matmul(pslv[:], hTs[kk][:], wlve_t[kk][:],
                                     start=(kk == 0), stop=(kk == 1))
                xr2 = fin.tile([128, 260], bf16, tag="xr2")
                nc.scalar.copy(xr2[:, 0:128], psmu[:, 0:128])
                nc.scalar.copy(xr2[:, 128:256], pslv[:, 0:128])
                ssv = xr2[:, 256:260].bitcast(f32)
                nc.vector.tensor_copy(ssv[:, 0:1], psmu[:, 128:129])
                nc.vector.tensor_copy(ssv[:, 1:2], pslv[:, 128:129])
                nc.sync.dma_start(
                    agin[b * 128:(b + 1) * 128, 0:260], xr2[:])
                sd2 = sm.tile([128, 2], f32, tag="sd2")
                nc.vector.tensor_copy(sd2[:, 0:1], psmu[:, 129:130])
                nc.vector.tensor_copy(sd2[:, 1:2], pslv[:, 129:130])
                nc.sync.dma_start(
                    EXTD2[b * 128:(b + 1) * 128, 0:2], sd2[:])

            pend1 = {}
            blk_ps = {}
            for ci in range(NCHUNK if upto != "A" else 0):
                xrow = gx.tile([128, TPC, XW], bf16, tag="xrow")
                nc.gpsimd.dma_gather(
                    xrow[:], XPT[:], srcg_t[:, ci * 64:(ci + 1) * 64],
                    CHUNK, CHUNK, XW)
                extd = ge.tile([128, TPC, EXW_], f32, tag="extd")
                nc.gpsimd.dma_gather(
                    extd[:], EXTD1[:], dstl_t[:, ci * 64:(ci + 1) * 64],
                    CHUNK, CHUNK, EXW_)
                # alpha for the whole chunk: z=ss+sd, leaky
                z = sm.tile([128, TPC, 4], f32, tag="z")
                nc.vector.tensor_tensor(z[:], xrow[:, :, 256:264].bitcast(f32),
                                        extd[:, :, 0:4], op=Alu.add)
                nc.vector.scalar_tensor_tensor(
                    z[:], in0=z[:], scalar=NEG, in1=z[:],
                    op0=Alu.mult, op1=Alu.max)
                # unweighted exp for denominators (f32)
                ex = sm.tile([128, TPC, 4], f32, tag="ex")
                nc.scalar.activation(ex[:], z[:], Act.Exp)
                # zw = leaky(z) + ln(w); exp via ACT expanded to a packed
                # per-column scale tile, then ONE packed DVE multiply (the
                # stride-0 broadcast form runs at half DVE rate in hardware)
                zw = sm.tile([128, TPC, 4], f32, tag="zw")
                wb = lnwT_t[:, ci * TPC:(ci + 1) * TPC]
                nc.vector.tensor_tensor(
                    zw[:], z[:],
                    wb.rearrange("p (t o) -> p t o", o=1).to_broadcast(
                        [128, TPC, 4]), op=Alu.add)
                exwb = xw.tile([128, TPC, 4, 64], bf16, tag="exwb")
                nc.scalar.activation(
                    exwb[:],
                    zw[:].rearrange("p t (h o) -> p t h o", o=1)
                    .to_broadcast([128, TPC, 4, 64]), Act.Exp)
                nc.vector.tensor_tensor(
                    xrow[:, :, 0:256], xrow[:, :, 0:256],
                    exwb[:].rearrange("p t h c -> p t (h c)"), op=Alu.mult)
                # unweighted ex -> hi/lo bf16 denominator cols 256:264
                nc.vector.tensor_copy(xrow[:, :, 256:260], ex[:])
                nc.vector.tensor_tensor(xrow[:, :, 260:264], ex[:],
                                        xrow[:, :, 256:260], op=Alu.subtract)

                for tt in range(TPC):
                    t = ci * TPC + tt
                    if t >= TILES:
                        continue
                    b = tb[t]
                    k = tk[t]
                    if k == 0:
                        # flush deferred stage-2 of blocks done >= 2 ago so
                        # their PE work never head-of-line-blocks these matmuls
                        for bb in sorted(pend1):
                            if bb <= b - 2:
                                _l1_stage2(bb, pend1.pop(bb))
                        blk_ps[b] = ps2.tile([128, 264], f32, tag="blk",
                                             name="blkps")
                    ps = blk_ps[b]
                    ohx = ohp.tile([128, 128], bf16, tag="ohx")
                    nc.vector.tensor_scalar(
                        ohx[:], iota_t[:], dstoffT_t[:, t:t + 1], None,
                        Alu.is_equal)
                    nc.tensor.matmul(
                        ps[:, 0:264], ohx[:], xrow[:, tt, 0:264],
                        start=(k == 0), stop=(k == tpb[b] - 1))
                    if k == tpb[b] - 1:
                        # stage-1: normalize + ELU -> hb (DVE/ACT only)
                        den8 = sm.tile([128, 8], f32, tag="den8")
                        nc.vector.tensor_copy(den8[:], ps[:, 256:264])
                        den = sm.tile([128, 4], f32, tag="den")
                        nc.vector.tensor_tensor(den[:], den8[:, 0:4],
                                                den8[:, 4:8], op=Alu.add)
                        nc.vector.tensor_scalar_add(den[:], den[:], EPS)
                        rec = sm.tile([128, 4], f32, tag="rec")
                        nc.vector.reciprocal(rec[:], den[:])
                        hb = hbp.tile([128, 256], f32, tag="hb")
                        for h in range(H):
                            nc.vector.scalar_tensor_tensor(
                                hb[:, h * 64:(h + 1) * 64],
                                in0=ps[:, h * 64:(h + 1) * 64],
                                scalar=rec[:, h:h + 1],
                                in1=b1b_t[:, h * 64:(h + 1) * 64],
                                op0=Alu.mult, op1=Alu.add)
                        # ELU: h = max(z,0) + exp(min(z,0)) - 1
                        zm = fin.tile([128, 256], f32, tag="zm")
                        nc.vector.tensor_scalar_min(zm[:], hb[:], 0.0)
                        ez = fin.tile([128, 256], f32, tag="ez")
                        nc.scalar.activation(ez[:], zm[:], Act.Exp)
                        nc.vector.scalar_tensor_tensor(
                            hb[:], in0=hb[:], scalar=0.0, in1=ez[:],
                            op0=Alu.max, op1=Alu.add)
                        nc.vector.tensor_scalar_add(hb[:], hb[:], -1.0)
                        pend1[b] = hb
                        del blk_ps[b]
            for bb in sorted(pend1):
                _l1_stage2(bb, pend1.pop(bb))

            # ---- exchange layer-2/3 message rows ----
            if upto in ("AG", "full"):
                nc.gpsimd.collective_compute(
                    "AllGather", mybir.AluOpType.bypass,
                    replica_groups=[list(range(NC))],
                    ins=[agin.opt()], outs=[agout.opt()])

            # pre-gather ALL L2/3 dst logits; their DMA + the compaction
            # run under the AllGather, when every engine is otherwise idle
            if upto == "full":
                for ci in range(NCHUNK):
                    ep = ge2.tile([128, TPC, EXW_], f32, tag="extd_pre",
                                  name="extdpre", bufs=4)
                    nc.gpsimd.dma_gather(
                        ep[:], EXTD2[:], dstl_t[:, ci * 64:(ci + 1) * 64],
                        CHUNK, CHUNK, EXW_)
                    # compact sd2 into the resident tile; full staging tile
                    # recycles while the AllGather keeps the DMA engines idle
                    nc.vector.tensor_copy(extd2c[:, ci, :, :],
                                          ep[:, :, 0:2])

            # ---- L2/3 edge phase (mu and lv share gathers) ----
            blk2 = {}
            for ci in range(NCHUNK if upto == "full" else 0):
                xrow = gx.tile([128, TPC, X2W], bf16, tag="xrow2")
                nc.gpsimd.dma_gather(
                    xrow[:], agout[:], src2_t[:, ci * 64:(ci + 1) * 64],
                    CHUNK, CHUNK, X2W)
                z = sm.tile([128, TPC, 2], f32, tag="z2")
                nc.vector.tensor_tensor(z[:], xrow[:, :, 256:260].bitcast(f32),
                                        extd2c[:, ci, :, :], op=Alu.add)
                nc.vector.scalar_tensor_tensor(
                    z[:], in0=z[:], scalar=NEG, in1=z[:],
                    op0=Alu.mult, op1=Alu.max)
                ex = sm.tile([128, TPC, 2], f32, tag="ex2")
                nc.scalar.activation(ex[:], z[:], Act.Exp)
                zw = sm.tile([128, TPC, 2], f32, tag="zw2")
                wb = lnwT_t[:, ci * TPC:(ci + 1) * TPC]
                nc.vector.tensor_tensor(
                    zw[:], z[:],
                    wb.rearrange("p (t o) -> p t o", o=1).to_broadcast(
                        [128, TPC, 2]), op=Alu.add)
                exwb = xw.tile([128, TPC, 2, 128], bf16, tag="exwb")
                nc.scalar.activation(
                    exwb[:],
                    zw[:].rearrange("p t (l o) -> p t l o", o=1)
                    .to_broadcast([128, TPC, 2, 128]), Act.Exp)
                nc.vector.tensor_tensor(
                    xrow[:, :, 0:256], xrow[:, :, 0:256],
                    exwb[:].rearrange("p t l c -> p t (l c)"), op=Alu.mult)
                nc.vector.tensor_copy(xrow[:, :, 256:258], ex[:])
                nc.vector.tensor_tensor(xrow[:, :, 258:260], ex[:],
                                        xrow[:, :, 256:258], op=Alu.subtract)

                for tt in range(TPC):
                    t = ci * TPC + tt
                    if t >= TILES:
                        continue
                    b = tb[t]
                    k = tk[t]
                    if k == 0:
                        blk2[b] = ps2.tile([128, 260], f32, tag="blk",
                                           name="blk2ps")
                    ps2t = blk2[b]
                    ohx = ohp.tile([128, 128], bf16, tag="ohx")
                    nc.vector.tensor_scalar(
                        ohx[:], iota_t[:], dstoffT_t[:, t:t + 1], None,
                        Alu.is_equal)
                    nc.tensor.matmul(
                        ps2t[:, 0:260], ohx[:], xrow[:, tt, 0:260],
                        start=(k == 0), stop=(k == tpb[b] - 1))
                    if k == tpb[b] - 1:
                        den4 = sm.tile([128, 4], f32, tag="den4")
                        nc.vector.tensor_copy(den4[:], ps2t[:, 256:260])
                        den = sm.tile([128, 2], f32, tag="den2")
                        nc.vector.tensor_tensor(
                            den[:], den4[:, 0:2], den4[:, 2:4], op=Alu.add)
                        nc.vector.tensor_scalar_add(den[:], den[:], EPS)
                        rec = sm.tile([128, 2], f32, tag="rec2")
                        nc.vector.reciprocal(rec[:], den[:])
                        ob = fin.tile([128, 2, 128], f32, tag="ob", bufs=4)
                        for li, bias_t in enumerate((bmub_t, blvb_t)):
                            nc.vector.scalar_tensor_tensor(
                                ob[:, li, :],
                                in0=ps2t[:, li * 128:(li + 1) * 128],
                                scalar=rec[:, li:li + 1],
                                in1=bias_t[:], op0=Alu.mult, op1=Alu.add)
                        nc.sync.dma_start(
                            mulv_out[b * 128:(b + 1) * 128, :, :], ob[:])
                        del blk2[b]

    nc.compile()
    return nc


def _prep_inputs(plan, x, edge_index, edge_weight, W1, att1, b1, Wmu, attmu,
                 bmu, Wlv, attlv, blv):
    EPAD = plan["EPAD"]
    boff, tpb = plan["boff"], plan["tpb"]
    import ml_dtypes
    bf = ml_dtypes.bfloat16

    src = np.asarray(edge_index[0], np.int64)
    dst = np.asarray(edge_index[1], np.int64)
    w = np.asarray(edge_weight, np.float32)
    x = np.asarray(x, np.float32)

    # fused weights
    att1 = np.asarray(att1, np.float32)          # [H, 2*C1]
    W1 = np.asarray(W1, np.float32)
    Wsd1 = np.zeros((FIN, H), np.float32)
    for h in range(H):
        Wsd1[:, h] = W1[:, h * C1:(h + 1) * C1] @ att1[h, :C1]
    # replicated src-attention row for on-chip ss1 reduce
    att1repb = np.tile(att1[:, C1:].reshape(1, H * C1), (128, 1)).astype(bf)

    attmu = np.asarray(attmu, np.float32).reshape(-1)        # [2*LAT]
    attlv = np.asarray(attlv, np.float32).reshape(-1)
    Wmu = np.asarray(Wmu, np.float32)
    Wlv = np.asarray(Wlv, np.float32)
    wmue = np.concatenate(
        [Wmu, (Wmu @ attmu[LAT:])[:, None], (Wmu @ attmu[:LAT])[:, None]],
        axis=1).astype(bf)
    wlve = np.concatenate(
        [Wlv, (Wlv @ attlv[LAT:])[:, None], (Wlv @ attlv[:LAT])[:, None]],
        axis=1).astype(bf)

    xT = np.zeros((FIN, NPADA), np.float32)
    xT[:, :N] = x.T
    xTb = xT.astype(bf)
    b1b = np.tile(np.asarray(b1, np.float32)[None, :], (128, 1))
    bmub = np.tile(np.asarray(bmu, np.float32)[None, :], (128, 1))
    blvb = np.tile(np.asarray(blv, np.float32)[None, :], (128, 1))
    iota = np.tile(np.arange(128, dtype=np.float32)[None, :],
                   (128, 1)).astype(bf)
    ident = np.eye(128, dtype=np.float32)

    # sort edges by dst, bucket per core, pad per 128-node block to TPB tiles
    order = np.argsort(dst, kind="stable")
    ssrc, sdst, sw = src[order], dst[order], w[order]
    core_of = sdst // NOWN
    in_maps = []
    for c in range(NC):
        m = core_of == c
        cs, cd, cw = ssrc[m], sdst[m] - c * NOWN, sw[m]
        blk = cd // 128
        e_src = np.zeros(EPAD, np.int64)
        e_dstloc = np.zeros(EPAD, np.int64)
        e_dstoff = np.full(EPAD, -1.0, np.float32)
        e_w = np.zeros(EPAD, np.float32)
        for b in range(BLOCKS):
            bm = blk == b
            nbe = int(bm.sum())
            if nbe > tpb[b] * 128:
                raise RuntimeError(f"block overflow core {c} block {b}: {nbe}")
            o = boff[b] * 128
            e_src[o:o + nbe] = cs[bm]
            e_dstloc[o:o + nbe] = cd[bm]
            e_dstoff[o:o + nbe] = (cd[bm] - b * 128).astype(np.float32)
            e_w[o:o + nbe] = cw[bm]
        e_lnw = np.full(EPAD, -60.0, np.float32)
        real = e_dstoff >= 0
        e_lnw[real] = np.log(np.maximum(e_w[real], 1e-38))
        own = e_src // NOWN
        e_src2 = own * NLOC + (e_src - own * NOWN)
        xTown = np.zeros((FIN, NLOC), np.float32)
        xTown[:, :NOWN] = x.T[:, c * NOWN:(c + 1) * NOWN]
        in_maps.append({
            "xTb": xTb, "xTown": xTown, "w1b": W1.astype(bf),
            "att1repb": att1repb,
            "wsd_own": Wsd1, "wmue": wmue, "wlve": wlve, "b1b": b1b,
            "bmub": bmub, "blvb": blvb, "iota": iota, "ident": ident,
            "srcg": _wrap_idxs(e_src), "src2": _wrap_idxs(e_src2),
            "dstl": _wrap_idxs(e_dstloc),
            "dstoffT": _colmajor(e_dstoff),
            "lnwT": _colmajor(e_lnw),
        })
    return in_maps


def kernel(x, edge_index, edge_weight, W1, att1, b1, Wmu, attmu, bmu,
           Wlv, attlv, blv):
    from concourse.bass_utils import run_bass_kernel_spmd

    plan = _make_plan(edge_index)
    key = plan["tpb"]
    if key not in _cache:
        _cache[key] = _build_module(plan)
    nc = _cache[key]
    _cache["nc"] = nc
    in_maps = _prep_inputs(plan, x, edge_index, edge_weight, W1, att1, b1,
                           Wmu, attmu, bmu, Wlv, attlv, blv)
    r = run_bass_kernel_spmd(nc, in_maps, list(range(NC)))
    mu = np.zeros((N, LAT), np.float32)
    lv = np.zeros((N, LAT), np.float32)
    for c in range(NC):
        out = r.results[c]["mulv_out"]
        mu[c * NOWN:(c + 1) * NOWN] = out[:NOWN, 0]
        lv[c * NOWN:(c + 1) * NOWN] = out[:NOWN, 1]
    return (mu, lv)

